# revision 44
# baseline (speedup 1.0000x reference)
import sys

sys.path.insert(0, "/opt/trn_rl_repo")

import numpy as np

import concourse.bass as bass
import concourse.tile as tile
from concourse import bacc, mybir
from concourse.bass_utils import run_bass_kernel_spmd

N_CORES = 8
B_FULL, F = 16384, 1024
B_CORE = B_FULL // N_CORES
P = 128
N_TILES = B_CORE // P

_compiled = {}


def _build(beta1: float, beta2: float, n_tiles: int = N_TILES):
    b_core = n_tiles * P
    nc = bacc.Bacc(
        "TRN2",
        target_bir_lowering=False,
        debug=False,
        enable_asserts=False,
        num_devices=N_CORES,
    )
    f32 = mybir.dt.float32
    x_d = nc.dram_tensor("x", [b_core, F], f32, kind="ExternalInput").ap()
    wb_d = nc.dram_tensor("wb", [P, 4 * F], f32, kind="ExternalInput").ap()
    out_d = nc.dram_tensor("out", [b_core, F], f32, kind="ExternalOutput").ap()

    x_r = x_d.rearrange("(n p) f -> n p f", p=P)
    out_r = out_d.rearrange("(n p) f -> n p f", p=P)

    AT = mybir.AluOpType

    with tile.TileContext(nc) as tc:
        with (
            tc.tile_pool(name="params", bufs=1) as params,
            tc.tile_pool(name="xp", bufs=4) as xp,
            tc.tile_pool(name="junk", bufs=2) as junkp,
            tc.tile_pool(name="small", bufs=4) as smallp,
            tc.tile_pool(name="outp", bufs=4) as outp,
        ):
            wb = params.tile([P, 4 * F], f32)
            nc.sync.dma_start(wb[:], wb_d[:])
            w = [wb[:, l * F : (l + 1) * F] for l in range(3)]
            b3 = wb[:, 3 * F : 4 * F]

            for i in range(n_tiles):
                x_t = xp.tile([P, F], f32)
                nc.sync.dma_start(x_t[:], x_r[i])

                a = smallp.tile([P, 3], f32, tag="a")
                junk = junkp.tile([P, F], f32)
                for l in range(3):
                    nc.vector.scalar_tensor_tensor(
                        out=junk[:],
                        in0=x_t[:],
                        scalar=1.0,
                        in1=w[l],
                        op0=AT.mult,
                        op1=AT.mult,
                        accum_out=a[:, l : l + 1],
                    )

                c1 = smallp.tile([P, 1], f32, tag="c1")
                nc.vector.tensor_scalar_add(c1[:], a[:, 0:1], 1.0)
                s1 = smallp.tile([P, 1], f32, tag="s1")
                nc.vector.tensor_scalar(
                    s1[:], a[:, 1:2], c1[:, 0:1], beta1, AT.mult, AT.add
                )
                c2 = smallp.tile([P, 1], f32, tag="c2")
                nc.vector.tensor_add(c2[:], c1[:], s1[:])
                s2 = smallp.tile([P, 1], f32, tag="s2")
                nc.vector.tensor_scalar(
                    s2[:], a[:, 2:3], c2[:, 0:1], beta2, AT.mult, AT.add
                )
                c3 = smallp.tile([P, 1], f32, tag="c3")
                nc.vector.tensor_add(c3[:], c2[:], s2[:])

                o_t = outp.tile([P, F], f32)
                nc.vector.scalar_tensor_tensor(
                    out=o_t[:], in0=x_t[:], scalar=c3[:, 0:1], in1=b3,
                    op0=AT.mult, op1=AT.add,
                )
                nc.scalar.dma_start(out_r[i], o_t[:])

    nc.compile()
    return nc


def _build_pe(beta1: float, beta2: float, n_tiles: int = N_TILES):
    b_core = n_tiles * P
    nc = bacc.Bacc(
        "TRN2",
        target_bir_lowering=False,
        debug=False,
        enable_asserts=False,
        num_devices=N_CORES,
    )
    f32 = mybir.dt.float32
    x_d = nc.dram_tensor("x", [b_core, F], f32, kind="ExternalInput").ap()
    aux_d = nc.dram_tensor("aux", [P, 162], f32, kind="ExternalInput").ap()
    b3_d = nc.dram_tensor("b3row", [1, F], f32, kind="ExternalInput").ap()
    out_d = nc.dram_tensor("out", [b_core, F], f32, kind="ExternalOutput").ap()

    x_r = x_d.rearrange("(n p) f -> n p f", p=P)
    out_r = out_d.rearrange("(n p) f -> n p f", p=P)

    AT = mybir.AluOpType
    AF = mybir.ActivationFunctionType

    with tile.TileContext(nc) as tc:
        with (
            tc.tile_pool(name="params", bufs=1) as params,
            tc.tile_pool(name="xp", bufs=4) as xp,
            tc.tile_pool(name="sbT", bufs=3) as sbTp,
            tc.tile_pool(name="psT", bufs=2, space="PSUM") as psTp,
            tc.tile_pool(name="psA", bufs=2, space="PSUM") as psAp,
            tc.tile_pool(name="small", bufs=4) as smallp,
            tc.tile_pool(name="outp", bufs=4) as outp,
        ):
            aux = params.tile([P, 162], f32)
            nc.sync.dma_start(aux[:], aux_d[:])
            ident = aux[:, 0:128]
            wsb = aux[:, 128:160]
            betas = aux[:, 160:162]

            b3s = params.tile([1, F], f32, tag="b3s")
            nc.sync.dma_start(b3s[:], b3_d[:])
            ones = params.tile([1, P], f32, tag="ones")
            nc.vector.memset(ones[:], 1.0)
            b3rep = params.tile([P, F], f32, tag="b3rep")
            for j in range(2):
                pb = psAp.tile([P, 512], f32, tag="pb")
                nc.tensor.matmul(
                    pb[:], ones[0:1, :], b3s[0:1, j * 512 : (j + 1) * 512],
                    start=True, stop=True,
                )
                nc.scalar.copy(b3rep[:, j * 512 : (j + 1) * 512], pb[:])

            for i in range(n_tiles):
                x_t = xp.tile([P, F], f32)
                nc.sync.dma_start(x_t[:], x_r[i])

                psT = psTp.tile([P, F], f32)
                for k in range(8):
                    nc.tensor.transpose(
                        psT[:, k * P : (k + 1) * P],
                        x_t[:, k * P : (k + 1) * P],
                        ident,
                    )
                sbT = sbTp.tile([P, F], f32)
                nc.scalar.copy(sbT[:], psT[:])

                psA = psAp.tile([P, 3], f32, tag="a")
                for k in range(8):
                    nc.tensor.matmul(
                        psA[:],
                        sbT[:, k * P : (k + 1) * P],
                        wsb[:, 4 * k : 4 * k + 3],
                        start=(k == 0),
                        stop=(k == 7),
                    )

                c1 = smallp.tile([P, 1], f32, tag="c1")
                nc.scalar.activation(c1[:], psA[:, 0:1], AF.Identity, bias=1.0)
                s1 = smallp.tile([P, 1], f32, tag="s1")
                nc.scalar.activation(
                    s1[:], psA[:, 1:2], AF.Identity,
                    bias=betas[:, 0:1], scale=c1[:, 0:1],
                )
                c2 = smallp.tile([P, 1], f32, tag="c2")
                nc.scalar.activation(
                    c2[:], c1[:], AF.Identity, bias=s1[:, 0:1]
                )
                s2 = smallp.tile([P, 1], f32, tag="s2")
                nc.scalar.activation(
                    s2[:], psA[:, 2:3], AF.Identity,
                    bias=betas[:, 1:2], scale=c2[:, 0:1],
                )
                c3 = smallp.tile([P, 1], f32, tag="c3")
                nc.scalar.activation(
                    c3[:], c2[:], AF.Identity, bias=s2[:, 0:1]
                )

                o_t = outp.tile([P, F], f32)
                nc.vector.scalar_tensor_tensor(
                    out=o_t[:], in0=x_t[:], scalar=c3[:, 0:1], in1=b3rep[:],
                    op0=AT.mult, op1=AT.add,
                )
                nc.scalar.dma_start(out_r[i], o_t[:])

    nc.compile()
    return nc


def _build_hybrid(beta1: float, beta2: float, n_tiles: int = N_TILES, n_pe: int = 6):
    b_core = n_tiles * P
    nc = bacc.Bacc(
        "TRN2",
        target_bir_lowering=False,
        debug=False,
        enable_asserts=False,
        num_devices=N_CORES,
    )
    f32 = mybir.dt.float32
    x_d = nc.dram_tensor("x", [b_core, F], f32, kind="ExternalInput").ap()
    aux_d = nc.dram_tensor("aux", [P, 162], f32, kind="ExternalInput").ap()
    b3_d = nc.dram_tensor("b3row", [1, F], f32, kind="ExternalInput").ap()
    wrep_d = nc.dram_tensor("wrep", [P, 3 * F], f32, kind="ExternalInput").ap()
    out_d = nc.dram_tensor("out", [b_core, F], f32, kind="ExternalOutput").ap()

    x_r = x_d.rearrange("(n p) f -> n p f", p=P)
    out_r = out_d.rearrange("(n p) f -> n p f", p=P)

    AT = mybir.AluOpType
    AF = mybir.ActivationFunctionType

    pe_set = {i for i in range(n_tiles) if (i + 1) * n_pe // n_tiles > i * n_pe // n_tiles}

    with tile.TileContext(nc) as tc:
        with (
            tc.tile_pool(name="params", bufs=1) as params,
            tc.tile_pool(name="xp", bufs=4) as xp,
            tc.tile_pool(name="junk", bufs=2) as junkp,
            tc.tile_pool(name="sbT", bufs=3) as sbTp,
            tc.tile_pool(name="psT", bufs=2, space="PSUM") as psTp,
            tc.tile_pool(name="psA", bufs=2, space="PSUM") as psAp,
            tc.tile_pool(name="small", bufs=4) as smallp,
            tc.tile_pool(name="outp", bufs=4) as outp,
        ):
            aux = params.tile([P, 162], f32)
            nc.sync.dma_start(aux[:], aux_d[:])
            ident = aux[:, 0:128]
            wsb = aux[:, 128:160]
            betas = aux[:, 160:162]

            wrep = params.tile([P, 3 * F], f32, tag="wrep")
            nc.sync.dma_start(wrep[:], wrep_d[:])
            wv = [wrep[:, l * F : (l + 1) * F] for l in range(3)]

            b3s = params.tile([1, F], f32, tag="b3s")
            nc.sync.dma_start(b3s[:], b3_d[:])
            ones = params.tile([1, P], f32, tag="ones")
            nc.vector.memset(ones[:], 1.0)
            b3rep = params.tile([P, F], f32, tag="b3rep")
            for j in range(2):
                pb = psAp.tile([P, 512], f32, tag="pb")
                nc.tensor.matmul(
                    pb[:], ones[0:1, :], b3s[0:1, j * 512 : (j + 1) * 512],
                    start=True, stop=True,
                )
                nc.scalar.copy(b3rep[:, j * 512 : (j + 1) * 512], pb[:])

            def recurrence(a_ap):
                c1 = smallp.tile([P, 1], f32, tag="c1")
                nc.scalar.activation(c1[:], a_ap[:, 0:1], AF.Identity, bias=1.0)
                s1 = smallp.tile([P, 1], f32, tag="s1")
                nc.scalar.activation(
                    s1[:], a_ap[:, 1:2], AF.Identity,
                    bias=betas[:, 0:1], scale=c1[:, 0:1],
                )
                c2 = smallp.tile([P, 1], f32, tag="c2")
                nc.scalar.activation(c2[:], c1[:], AF.Identity, bias=s1[:, 0:1])
                s2 = smallp.tile([P, 1], f32, tag="s2")
                nc.scalar.activation(
                    s2[:], a_ap[:, 2:3], AF.Identity,
                    bias=betas[:, 1:2], scale=c2[:, 0:1],
                )
                c3 = smallp.tile([P, 1], f32, tag="c3")
                nc.scalar.activation(c3[:], c2[:], AF.Identity, bias=s2[:, 0:1])
                return c3

            for i in range(n_tiles):
                x_t = xp.tile([P, F], f32)
                nc.sync.dma_start(x_t[:], x_r[i])

                if i in pe_set:
                    psT = psTp.tile([P, F], f32)
                    for k in range(8):
                        nc.tensor.transpose(
                            psT[:, k * P : (k + 1) * P],
                            x_t[:, k * P : (k + 1) * P],
                            ident,
                        )
                    sbT = sbTp.tile([P, F], f32)
                    nc.scalar.copy(sbT[:], psT[:])
                    psA = psAp.tile([P, 3], f32, tag="a")
                    for k in range(8):
                        nc.tensor.matmul(
                            psA[:],
                            sbT[:, k * P : (k + 1) * P],
                            wsb[:, 4 * k : 4 * k + 3],
                            start=(k == 0),
                            stop=(k == 7),
                        )
                    c3 = recurrence(psA)
                else:
                    a = smallp.tile([P, 3], f32, tag="adve")
                    junk = junkp.tile([P, F], f32)
                    for l in range(3):
                        nc.vector.scalar_tensor_tensor(
                            out=junk[:], in0=x_t[:], scalar=1.0, in1=wv[l],
                            op0=AT.mult, op1=AT.mult,
                            accum_out=a[:, l : l + 1],
                        )
                    c3 = recurrence(a)

                o_t = outp.tile([P, F], f32)
                nc.vector.scalar_tensor_tensor(
                    out=o_t[:], in0=x_t[:], scalar=c3[:, 0:1], in1=b3rep[:],
                    op0=AT.mult, op1=AT.add,
                )
                nc.scalar.dma_start(out_r[i], o_t[:])

    nc.compile()
    return nc


def _build_h2(beta1: float, beta2: float, n_tiles: int = N_TILES, n_pe: int = 12):
    b_core = n_tiles * P
    assert n_tiles % 4 == 0
    nc = bacc.Bacc(
        "TRN2",
        target_bir_lowering=False,
        debug=False,
        enable_asserts=False,
        num_devices=N_CORES,
    )
    f32 = mybir.dt.float32
    f32r = mybir.dt.float32r
    x_d = nc.dram_tensor("x", [b_core, F], f32, kind="ExternalInput").ap()
    aux_d = nc.dram_tensor("aux", [P, 162], f32, kind="ExternalInput").ap()
    b3_d = nc.dram_tensor("b3row", [1, F], f32, kind="ExternalInput").ap()
    w3_d = nc.dram_tensor("w3row", [3, F], f32, kind="ExternalInput").ap()
    out_d = nc.dram_tensor("out", [b_core, F], f32, kind="ExternalOutput").ap()

    x_r = x_d.rearrange("(n p) f -> n p f", p=P)
    out_r = out_d.rearrange("(n p) f -> n p f", p=P)

    AT = mybir.AluOpType

    pe_set = {i for i in range(n_tiles) if (i + 1) * n_pe // n_tiles > i * n_pe // n_tiles}

    with tile.TileContext(nc) as tc:
        with (
            tc.tile_pool(name="params", bufs=1) as params,
            tc.tile_pool(name="xp", bufs=16) as xp,
            tc.tile_pool(name="junk", bufs=3) as junkp,
            tc.tile_pool(name="sbT", bufs=3) as sbTp,
            tc.tile_pool(name="psT", bufs=2, space="PSUM") as psTp,
            tc.tile_pool(name="psA", bufs=2, space="PSUM") as psAp,
            tc.tile_pool(name="psB", bufs=2, space="PSUM") as psBp,
            tc.tile_pool(name="small", bufs=2) as smallp,
            tc.tile_pool(name="outp", bufs=10) as outp,
        ):
            aux = params.tile([P, 162], f32)
            nc.sync.dma_start(aux[:], aux_d[:])
            ident = aux[:, 0:128]
            wsb = aux[:, 128:160]

            b3s = params.tile([1, F], f32, tag="b3s")
            nc.sync.dma_start(b3s[:], b3_d[:])
            w3s = []
            for l in range(3):
                t = params.tile([1, F], f32, tag=f"w3s{l}")
                nc.sync.dma_start(t[:], w3_d[l : l + 1, :])
                w3s.append(t)
            ones = params.tile([1, P], f32, tag="ones")
            nc.vector.memset(ones[:], 1.0)
            wv = []
            for l in range(3):
                wrep_l = params.tile([P, F], f32, tag=f"w{l}rep", name=f"w{l}rep")
                wv.append(wrep_l[:])
            b3rep = params.tile([P, F], f32, tag="b3rep")
            bcasts = [(wv[l], w3s[l][0:1, :]) for l in range(3)]
            bcasts.append((b3rep[:], b3s[0:1, :]))
            for dst, src in bcasts:
                for j in range(2):
                    pb = psBp.tile([P, 512], f32, tag="pb")
                    nc.tensor.matmul(
                        pb[:], ones[0:1, :], src[:, j * 512 : (j + 1) * 512],
                        start=True, stop=True,
                    )
                    nc.scalar.copy(dst[:, j * 512 : (j + 1) * 512], pb[:])

            def dve_recurrence(a_grp, c3g, width):
                av = a_grp[:, 0 : 3 * width].rearrange("p (j l) -> p j l", l=3)
                a0, a1, a2 = av[:, :, 0], av[:, :, 1], av[:, :, 2]
                c1 = smallp.tile([P, 4], f32, tag="c1")
                nc.vector.tensor_scalar_add(c1[:, 0:width], a0, 1.0)
                s1p = smallp.tile([P, 4], f32, tag="s1p")
                nc.vector.scalar_tensor_tensor(
                    out=s1p[:, 0:width], in0=a1, scalar=1.0, in1=c1[:, 0:width],
                    op0=AT.mult, op1=AT.mult,
                )
                c2 = smallp.tile([P, 4], f32, tag="c2")
                nc.vector.scalar_tensor_tensor(
                    out=c2[:, 0:width], in0=c1[:, 0:width], scalar=beta1,
                    in1=s1p[:, 0:width], op0=AT.add, op1=AT.add,
                )
                s2p = smallp.tile([P, 4], f32, tag="s2p")
                nc.vector.scalar_tensor_tensor(
                    out=s2p[:, 0:width], in0=a2, scalar=1.0, in1=c2[:, 0:width],
                    op0=AT.mult, op1=AT.mult,
                )
                nc.vector.scalar_tensor_tensor(
                    out=c3g[:, 0:width], in0=c2[:, 0:width], scalar=beta2,
                    in1=s2p[:, 0:width], op0=AT.add, op1=AT.add,
                )

            def act_recurrence(psA, betas):
                AF = mybir.ActivationFunctionType
                c1 = smallp.tile([P, 1], f32, tag="pc1")
                nc.scalar.activation(c1[:], psA[:, 0:1], AF.Identity, bias=1.0)
                s1 = smallp.tile([P, 1], f32, tag="ps1")
                nc.scalar.activation(
                    s1[:], psA[:, 1:2], AF.Identity,
                    bias=betas[:, 0:1], scale=c1[:, 0:1],
                )
                c2 = smallp.tile([P, 1], f32, tag="pc2")
                nc.scalar.activation(c2[:], c1[:], AF.Identity, bias=s1[:, 0:1])
                s2 = smallp.tile([P, 1], f32, tag="ps2")
                nc.scalar.activation(
                    s2[:], psA[:, 2:3], AF.Identity,
                    bias=betas[:, 1:2], scale=c2[:, 0:1],
                )
                c3 = smallp.tile([P, 1], f32, tag="pc3")
                nc.scalar.activation(c3[:], c2[:], AF.Identity, bias=s2[:, 0:1])
                return c3

            betas = aux[:, 160:162]
            dve_grp = []
            a_grp = None
            c3g = None

            def flush_dve_group():
                nonlocal dve_grp, a_grp, c3g
                if not dve_grp:
                    return
                dve_recurrence(a_grp, c3g, len(dve_grp))
                for j, (i, x_t) in enumerate(dve_grp):
                    o_t = outp.tile([P, F], f32)
                    nc.vector.scalar_tensor_tensor(
                        out=o_t[:], in0=x_t[:], scalar=c3g[:, j : j + 1],
                        in1=b3rep[:], op0=AT.mult, op1=AT.add,
                    )
                    nc.scalar.dma_start(out_r[i], o_t[:])
                dve_grp = []
                a_grp = None
                c3g = None

            for i in range(n_tiles):
                x_t = xp.tile([P, F], f32)
                nc.sync.dma_start(x_t[:], x_r[i])

                if i in pe_set:
                    psT = psTp.tile([P, F], f32)
                    for k in range(8):
                        nc.tensor.transpose(
                            psT[:, k * P : (k + 1) * P],
                            x_t[:, k * P : (k + 1) * P],
                            ident,
                        )
                    sbT = sbTp.tile([P, F], f32)
                    nc.scalar.copy(sbT[:], psT[:])
                    psA = psAp.tile([P, 3], f32, tag="a")
                    for k in range(8):
                        nc.tensor.matmul(
                            psA[:],
                            sbT[:, k * P : (k + 1) * P],
                            wsb[:, 4 * k : 4 * k + 3],
                            start=(k == 0),
                            stop=(k == 7),
                        )
                    c3 = act_recurrence(psA, betas)
                    o_t = outp.tile([P, F], f32)
                    nc.vector.scalar_tensor_tensor(
                        out=o_t[:], in0=x_t[:], scalar=c3[:, 0:1],
                        in1=b3rep[:], op0=AT.mult, op1=AT.add,
                    )
                    nc.scalar.dma_start(out_r[i], o_t[:])
                else:
                    if not dve_grp:
                        a_grp = smallp.tile([P, 12], f32, tag="ag")
                        c3g = smallp.tile([P, 4], f32, tag="c3g")
                    j = len(dve_grp)
                    junk = junkp.tile([P, F], f32)
                    for l in range(3):
                        nc.vector.scalar_tensor_tensor(
                            out=junk[:], in0=x_t[:], scalar=1.0, in1=wv[l],
                            op0=AT.mult, op1=AT.mult,
                            accum_out=a_grp[:, 3 * j + l : 3 * j + l + 1],
                        )
                    dve_grp.append((i, x_t))
                    if len(dve_grp) == 4:
                        flush_dve_group()
            flush_dve_group()

    nc.compile()
    return nc


def _build_b16(beta1: float, beta2: float, n_tiles: int = N_TILES, n_pe: int = 6):
    b_core = n_tiles * P
    assert n_tiles % 4 == 0
    nc = bacc.Bacc(
        "TRN2",
        target_bir_lowering=False,
        debug=False,
        enable_asserts=False,
        num_devices=N_CORES,
    )
    f32 = mybir.dt.float32
    bf16 = mybir.dt.bfloat16
    x_d = nc.dram_tensor("x", [b_core, F], bf16, kind="ExternalInput").ap()
    aux_d = nc.dram_tensor("auxb", [P, 160], bf16, kind="ExternalInput").ap()
    b3_d = nc.dram_tensor("b3row", [1, F], bf16, kind="ExternalInput").ap()
    w3_d = nc.dram_tensor("w3row", [3, F], bf16, kind="ExternalInput").ap()
    bt_d = nc.dram_tensor("betas", [P, 2], f32, kind="ExternalInput").ap()
    out_d = nc.dram_tensor("out", [b_core, F], bf16, kind="ExternalOutput").ap()

    x_r2 = x_d.rearrange("(n two p) f -> n p two f", two=2, p=P)
    out_r2 = out_d.rearrange("(n two p) f -> n p two f", two=2, p=P)

    AT = mybir.AluOpType
    AF = mybir.ActivationFunctionType

    pe_set = {i for i in range(n_tiles) if (i + 1) * n_pe // n_tiles > i * n_pe // n_tiles}

    with tile.TileContext(nc) as tc:
        with (
            tc.tile_pool(name="params", bufs=1) as params,
            tc.tile_pool(name="xp", bufs=8) as xp,
            tc.tile_pool(name="junk", bufs=3) as junkp,
            tc.tile_pool(name="sbT", bufs=3) as sbTp,
            tc.tile_pool(name="psT", bufs=2, space="PSUM") as psTp,
            tc.tile_pool(name="psA", bufs=2, space="PSUM") as psAp,
            tc.tile_pool(name="psB", bufs=2, space="PSUM") as psBp,
            tc.tile_pool(name="small", bufs=2) as smallp,
            tc.tile_pool(name="outp", bufs=6) as outp,
        ):
            aux = params.tile([P, 160], bf16)
            nc.sync.dma_start(aux[:], aux_d[:])
            ident = aux[:, 0:128]
            wsb = aux[:, 128:160]
            betas = params.tile([P, 2], f32, tag="betas")
            nc.sync.dma_start(betas[:], bt_d[:])

            b3s = params.tile([1, F], bf16, tag="b3s")
            nc.sync.dma_start(b3s[:], b3_d[:])
            w3s = []
            for l in range(3):
                t = params.tile([1, F], bf16, tag=f"w3s{l}")
                nc.sync.dma_start(t[:], w3_d[l : l + 1, :])
                w3s.append(t)
            ones = params.tile([1, P], bf16, tag="ones")
            nc.vector.memset(ones[:], 1.0)
            wv = []
            for l in range(3):
                wrep_l = params.tile([P, F], bf16, tag=f"w{l}rep", name=f"w{l}rep")
                wv.append(wrep_l[:])
            b3rep = params.tile([P, F], bf16, tag="b3rep")
            bcasts = [(wv[l], w3s[l][0:1, :]) for l in range(3)]
            bcasts.append((b3rep[:], b3s[0:1, :]))
            for dst, src in bcasts:
                for j in range(2):
                    pb = psBp.tile([P, 512], f32, tag="pb")
                    nc.tensor.matmul(
                        pb[:], ones[0:1, :], src[:, j * 512 : (j + 1) * 512],
                        start=True, stop=True,
                    )
                    nc.scalar.copy(dst[:, j * 512 : (j + 1) * 512], pb[:])

            def dve_recurrence(a_grp, c3g, width):
                av = a_grp[:, 0 : 3 * width].rearrange("p (j l) -> p j l", l=3)
                a0, a1, a2 = av[:, :, 0], av[:, :, 1], av[:, :, 2]
                c1 = smallp.tile([P, 4], f32, tag="c1")
                nc.vector.tensor_scalar_add(c1[:, 0:width], a0, 1.0)
                s1p = smallp.tile([P, 4], f32, tag="s1p")
                nc.vector.scalar_tensor_tensor(
                    out=s1p[:, 0:width], in0=a1, scalar=1.0, in1=c1[:, 0:width],
                    op0=AT.mult, op1=AT.mult,
                )
                c2 = smallp.tile([P, 4], f32, tag="c2")
                nc.vector.scalar_tensor_tensor(
                    out=c2[:, 0:width], in0=c1[:, 0:width], scalar=beta1,
                    in1=s1p[:, 0:width], op0=AT.add, op1=AT.add,
                )
                s2p = smallp.tile([P, 4], f32, tag="s2p")
                nc.vector.scalar_tensor_tensor(
                    out=s2p[:, 0:width], in0=a2, scalar=1.0, in1=c2[:, 0:width],
                    op0=AT.mult, op1=AT.mult,
                )
                nc.vector.scalar_tensor_tensor(
                    out=c3g[:, 0:width], in0=c2[:, 0:width], scalar=beta2,
                    in1=s2p[:, 0:width], op0=AT.add, op1=AT.add,
                )

            def act_recurrence(psA):
                c1 = smallp.tile([P, 1], f32, tag="pc1")
                nc.scalar.activation(c1[:], psA[:, 0:1], AF.Identity, bias=1.0)
                s1 = smallp.tile([P, 1], f32, tag="ps1")
                nc.scalar.activation(
                    s1[:], psA[:, 1:2], AF.Identity,
                    bias=betas[:, 0:1], scale=c1[:, 0:1],
                )
                c2 = smallp.tile([P, 1], f32, tag="pc2")
                nc.scalar.activation(c2[:], c1[:], AF.Identity, bias=s1[:, 0:1])
                s2 = smallp.tile([P, 1], f32, tag="ps2")
                nc.scalar.activation(
                    s2[:], psA[:, 2:3], AF.Identity,
                    bias=betas[:, 1:2], scale=c2[:, 0:1],
                )
                c3 = smallp.tile([P, 1], f32, tag="pc3")
                nc.scalar.activation(c3[:], c2[:], AF.Identity, bias=s2[:, 0:1])
                return c3

            xpairs = []
            for j in range(n_tiles // 2):
                x2 = xp.tile([P, 2 * F], bf16, tag="x2")
                nc.sync.dma_start(
                    x2[:].rearrange("p (two f) -> p two f", two=2), x_r2[j]
                )
                xpairs.append(x2)

            def x_tile(i):
                return xpairs[i // 2][:, (i % 2) * F : (i % 2 + 1) * F]

            opairs = {}

            def emit_out(i, c3_ap):
                j, h = i // 2, i % 2
                if j not in opairs:
                    opairs[j] = outp.tile([P, 2 * F], bf16, tag="o2", name="o2")
                o2 = opairs[j]
                nc.vector.scalar_tensor_tensor(
                    out=o2[:, h * F : (h + 1) * F], in0=x_tile(i), scalar=c3_ap,
                    in1=b3rep[:], op0=AT.mult, op1=AT.add,
                )
                done = opairs.get(("done", j), 0) + 1
                opairs[("done", j)] = done
                if done == 2:
                    nc.scalar.dma_start(
                        out_r2[j], o2[:].rearrange("p (two f) -> p two f", two=2)
                    )

            dve_grp = []
            a_grp = None
            c3g = None

            def flush_dve_group():
                nonlocal dve_grp, a_grp, c3g
                if not dve_grp:
                    return
                dve_recurrence(a_grp, c3g, len(dve_grp))
                for j, i in enumerate(dve_grp):
                    emit_out(i, c3g[:, j : j + 1])
                dve_grp = []
                a_grp = None
                c3g = None

            for i in range(n_tiles):
                x_t = x_tile(i)
                if i in pe_set:
                    psT = psTp.tile([P, F], bf16)
                    for k in range(8):
                        nc.tensor.transpose(
                            psT[:, k * P : (k + 1) * P],
                            x_t[:, k * P : (k + 1) * P],
                            ident,
                        )
                    sbT = sbTp.tile([P, F], bf16)
                    nc.scalar.copy(sbT[:], psT[:])
                    psA = psAp.tile([P, 3], f32, tag="a")
                    for k in range(8):
                        nc.tensor.matmul(
                            psA[:],
                            sbT[:, k * P : (k + 1) * P],
                            wsb[:, 4 * k : 4 * k + 3],
                            start=(k == 0),
                            stop=(k == 7),
                        )
                    c3 = act_recurrence(psA)
                    emit_out(i, c3[:, 0:1])
                else:
                    if not dve_grp:
                        a_grp = smallp.tile([P, 12], f32, tag="ag")
                        c3g = smallp.tile([P, 4], f32, tag="c3g")
                    j = len(dve_grp)
                    junk = junkp.tile([P, F], bf16)
                    for l in range(3):
                        nc.vector.scalar_tensor_tensor(
                            out=junk[:], in0=x_t[:], scalar=1.0, in1=wv[l],
                            op0=AT.mult, op1=AT.mult,
                            accum_out=a_grp[:, 3 * j + l : 3 * j + l + 1],
                        )
                    dve_grp.append(i)
                    if len(dve_grp) == 4:
                        flush_dve_group()
            flush_dve_group()

    nc.compile()
    return nc


def _prep_pe_inputs(x, kernels, bias):
    x = np.ascontiguousarray(x, dtype=np.float32)
    W = np.ascontiguousarray(kernels[:, :, 0], dtype=np.float32)
    Bb = np.ascontiguousarray(bias[:, :, 0], dtype=np.float32)
    beta1 = float(Bb[0] @ W[1])
    beta2 = float((Bb[0] + Bb[1]) @ W[2])
    b3 = (Bb[0] + Bb[1] + Bb[2]).astype(np.float32)

    aux = np.zeros((P, 162), dtype=np.float32)
    aux[:, 0:128] = np.eye(P, dtype=np.float32)
    wsb4 = np.zeros((P, 8, 4), dtype=np.float32)
    wsb4[:, :, 0:3] = W.T.reshape(8, P, 3).transpose(1, 0, 2)
    aux[:, 128:160] = wsb4.reshape(P, 32)
    aux[:, 160] = beta1
    aux[:, 161] = beta2
    b3row = b3.reshape(1, F)
    return x, aux, b3row, beta1, beta2


def _build_tp(beta1: float, beta2: float, n_blocks: int = 5):
    if n_blocks == 5:
        BLKS = [128, 384, 512, 512, 512]
    else:
        BLKS = [B_CORE // n_blocks] * n_blocks
    assert sum(BLKS) == B_CORE and all(b % P == 0 for b in BLKS)
    OFFS = [sum(BLKS[:i]) for i in range(len(BLKS))]
    NCH = F // P
    nc = bacc.Bacc(
        "TRN2",
        target_bir_lowering=False,
        debug=False,
        enable_asserts=False,
        num_devices=N_CORES,
    )
    f32 = mybir.dt.float32
    bf16 = mybir.dt.bfloat16
    xt_d = nc.dram_tensor("xt", [F, B_CORE], bf16, kind="ExternalInput").ap()
    prmb_d = nc.dram_tensor("prmb", [P, 32], bf16, kind="ExternalInput").ap()
    prmf_d = nc.dram_tensor("prmf", [P, 140], f32, kind="ExternalInput").ap()
    outt_d = nc.dram_tensor("outt", [F, B_CORE], bf16, kind="ExternalOutput").ap()

    def io_view(dram, g):
        return dram[:, OFFS[g] : OFFS[g] + BLKS[g]].rearrange(
            "(h k p) b -> h p k b", h=2, k=NCH // 2, p=P
        )

    AT = mybir.AluOpType
    AF = mybir.ActivationFunctionType

    with tile.TileContext(nc) as tc:
        with (
            tc.tile_pool(name="params", bufs=1) as params,
            tc.tile_pool(name="xp", bufs=4) as xp,
            tc.tile_pool(name="asb", bufs=3) as asbp,
            tc.tile_pool(name="smalls", bufs=3) as smallp,
            tc.tile_pool(name="c3rp", bufs=3) as c3rp,
            tc.tile_pool(name="tmp", bufs=3) as tmpp,
            tc.tile_pool(name="otp", bufs=4) as otp,
            tc.tile_pool(name="psA", bufs=2, space="PSUM") as psAp,
            tc.tile_pool(name="psAT", bufs=2, space="PSUM") as psATp,
            tc.tile_pool(name="psC", bufs=2, space="PSUM") as psCp,
            tc.tile_pool(name="psR", bufs=2, space="PSUM") as psRp,
        ):
            prmb = params.tile([P, 32], bf16)
            nc.sync.dma_start(prmb[:], prmb_d[:])
            wck = prmb[:, 0:32]
            prmf = params.tile([P, 140], f32, tag="prmf")
            nc.sync.dma_start(prmf[:], prmf_d[:])
            ident128 = prmf[:, 0:128]
            ident3 = prmf[0:3, 128:131]
            b3cols = prmf[:, 132:140]
            onesf = params.tile([1, P], f32, tag="onesf")
            nc.vector.memset(onesf[:], 1.0)

            BMAX = max(BLKS)
            b3rep = params.tile([P, NCH * BMAX], bf16, tag="b3rep")
            b3bc = (
                b3cols[:, 0:NCH]
                .rearrange("p (k o) -> p k o", o=1)
                .broadcast_to([P, NCH, BMAX])
            )
            nc.vector.tensor_copy(
                b3rep[:].rearrange("p (k b) -> p k b", k=NCH), b3bc
            )

            xtbs = {}
            psATs = {}
            c3blks = {}
            c3reps = {}
            tmps = {}

            def stage_load(g):
                B = BLKS[g]
                xtb = xp.tile([P, NCH * B], bf16, name="xtb")
                xv = xtb[:].rearrange("p (h k b) -> h p k b", h=2, b=B)
                src = io_view(xt_d, g)
                for h in range(2):
                    nc.sync.dma_start(xv[h], src[h])
                xtbs[g] = xtb

            def stage_dots(g):
                B = BLKS[g]
                xtb = xtbs[g]
                psA = psAp.tile([3, B], f32, tag="a", name="psA")
                for k in range(NCH):
                    nc.tensor.matmul(
                        psA[:],
                        wck[:, 4 * k : 4 * k + 3],
                        xtb[:, k * B : (k + 1) * B],
                        start=(k == 0),
                        stop=(k == NCH - 1),
                    )
                aSB = asbp.tile([3, B], f32, tag="asb", name="aSB")
                nc.scalar.copy(aSB[:], psA[:])
                T = B // P
                psAT = psATp.tile([P, 3 * T], f32, tag="at", name="psAT")
                for t in range(T):
                    nc.tensor.transpose(
                        psAT[:, 3 * t : 3 * t + 3],
                        aSB[:, t * P : (t + 1) * P],
                        ident3,
                    )
                psATs[g] = psAT

            def stage_rec(g):
                T = BLKS[g] // P
                psAT = psATs.pop(g)
                av = psAT[:].rearrange("p (t l) -> p t l", l=3)
                a0, a1, a2 = av[:, :, 0], av[:, :, 1], av[:, :, 2]
                c1 = smallp.tile([P, T], f32, tag="c1", name="c1")
                nc.vector.tensor_scalar_add(c1[:], a0, 1.0)
                s1p = smallp.tile([P, T], f32, tag="s1p", name="s1p")
                nc.vector.scalar_tensor_tensor(
                    out=s1p[:], in0=a1, scalar=1.0, in1=c1[:],
                    op0=AT.mult, op1=AT.mult,
                )
                c2 = smallp.tile([P, T], f32, tag="c2", name="c2")
                nc.vector.scalar_tensor_tensor(
                    out=c2[:], in0=c1[:], scalar=beta1, in1=s1p[:],
                    op0=AT.add, op1=AT.add,
                )
                s2p = smallp.tile([P, T], f32, tag="s2p", name="s2p")
                nc.vector.scalar_tensor_tensor(
                    out=s2p[:], in0=a2, scalar=1.0, in1=c2[:],
                    op0=AT.mult, op1=AT.mult,
                )
                c3blk = smallp.tile([P, T], f32, tag="c3b", name="c3blk")
                nc.vector.scalar_tensor_tensor(
                    out=c3blk[:], in0=c2[:], scalar=beta2, in1=s2p[:],
                    op0=AT.add, op1=AT.add,
                )
                c3blks[g] = c3blk

            def stage_bcast(g):
                B = BLKS[g]
                T = B // P
                c3blk = c3blks.pop(g)
                psc3T = psCp.tile([1, B], f32, tag="c3t", name="psc3T")
                for t in range(T):
                    nc.tensor.transpose(
                        psc3T[0:1, t * P : (t + 1) * P],
                        c3blk[:, t : t + 1],
                        ident128,
                    )
                c3Ts = smallp.tile([1, B], f32, tag="c3ts", name="c3Ts")
                nc.scalar.copy(c3Ts[:], psc3T[:])
                psR = psRp.tile([P, B], f32, tag="c3rep", name="psR")
                for t in range(T):
                    nc.tensor.matmul(
                        psR[:, t * P : (t + 1) * P],
                        onesf[0:1, :],
                        c3Ts[0:1, t * P : (t + 1) * P],
                        start=True,
                        stop=True,
                    )
                c3rep = c3rp.tile([P, B], bf16, tag="c3repb", name="c3rep")
                nc.scalar.copy(c3rep[:], psR[:])
                c3reps[g] = c3rep

            def stage_fin_mult(g):
                B = BLKS[g]
                xtb = xtbs[g]
                c3rep = c3reps.pop(g)
                tmp = tmpp.tile([P, NCH * B], bf16, tag="t", name="tmp")
                c3bc = (
                    c3rep[:]
                    .rearrange("p (o b) -> p o b", o=1)
                    .broadcast_to([P, NCH, B])
                )
                nc.vector.tensor_tensor(
                    out=tmp[:].rearrange("p (k b) -> p k b", k=NCH),
                    in0=xtb[:].rearrange("p (k b) -> p k b", k=NCH),
                    in1=c3bc,
                    op=AT.mult,
                )
                tmps[g] = tmp

            def stage_fin_add(g):
                B = BLKS[g]
                xtbs.pop(g)
                tmp = tmps.pop(g)
                otb = otp.tile([P, NCH * B], bf16, name="otb")
                ov = otb[:].rearrange("p (h k b) -> h p k b", h=2, b=B)
                dst = io_view(outt_d, g)
                b3v = b3rep[:].rearrange("p (k b) -> p k b", k=NCH)
                nc.vector.tensor_tensor(
                    out=otb[:].rearrange("p (k b) -> p k b", k=NCH)[:, 0:4],
                    in0=tmp[:].rearrange("p (k b) -> p k b", k=NCH)[:, 0:4],
                    in1=b3v[:, 0:4, 0:B],
                    op=AT.add,
                )
                nc.scalar.dma_start(dst[0], ov[0])
                for k in (4, 5, 6):
                    nc.scalar.activation(
                        otb[:, k * B : (k + 1) * B],
                        tmp[:, k * B : (k + 1) * B],
                        AF.Identity,
                        bias=b3cols[:, k : k + 1],
                    )
                nc.vector.tensor_scalar_add(
                    otb[:, 7 * B : 8 * B],
                    tmp[:, 7 * B : 8 * B],
                    b3cols[:, 7:8],
                )
                nc.scalar.dma_start(dst[1], ov[1])

            nb = len(BLKS)
            stage_load(0)
            stage_load(1)
            stage_dots(0)
            stage_rec(0)
            for g in range(nb):
                if g + 2 < nb:
                    stage_load(g + 2)
                stage_bcast(g)
                if g + 1 < nb:
                    stage_dots(g + 1)
                stage_fin_mult(g)
                if g + 1 < nb:
                    stage_rec(g + 1)
                stage_fin_add(g)

    nc.compile()
    return nc


def _prep_tp_inputs(x, kernels, bias):
    import ml_dtypes

    bf = ml_dtypes.bfloat16
    x_bf = np.ascontiguousarray(x, dtype=np.float32).astype(bf)
    W = np.ascontiguousarray(kernels[:, :, 0], dtype=np.float32)
    Bb = np.ascontiguousarray(bias[:, :, 0], dtype=np.float32)
    beta1 = float(Bb[0] @ W[1])
    beta2 = float((Bb[0] + Bb[1]) @ W[2])
    b3 = (Bb[0] + Bb[1] + Bb[2]).astype(np.float32)

    wck = np.zeros((P, 8, 4), dtype=np.float32)
    wck[:, :, 0:3] = W.T.reshape(8, P, 3).transpose(1, 0, 2)
    prmb = wck.reshape(P, 32).astype(bf)

    prmf = np.zeros((P, 140), dtype=np.float32)
    prmf[:, 0:128] = np.eye(P, dtype=np.float32)
    prmf[0:3, 128:131] = np.eye(3, dtype=np.float32)
    prmf[:, 132:140] = b3.reshape(8, P).T
    return x_bf, prmb, prmf, beta1, beta2


def _prep_b16_inputs(x, kernels, bias):
    import ml_dtypes

    bf = ml_dtypes.bfloat16
    x_bf = np.ascontiguousarray(x, dtype=np.float32).astype(bf)
    W = np.ascontiguousarray(kernels[:, :, 0], dtype=np.float32)
    Bb = np.ascontiguousarray(bias[:, :, 0], dtype=np.float32)
    beta1 = float(Bb[0] @ W[1])
    beta2 = float((Bb[0] + Bb[1]) @ W[2])
    b3 = (Bb[0] + Bb[1] + Bb[2]).astype(np.float32)

    auxb = np.zeros((P, 160), dtype=np.float32)
    auxb[:, 0:128] = np.eye(P, dtype=np.float32)
    wsb4 = np.zeros((P, 8, 4), dtype=np.float32)
    wsb4[:, :, 0:3] = W.T.reshape(8, P, 3).transpose(1, 0, 2)
    auxb[:, 128:160] = wsb4.reshape(P, 32)
    auxb = auxb.astype(bf)
    b3row = b3.reshape(1, F).astype(bf)
    w3row = W.astype(bf)
    betas = np.zeros((P, 2), dtype=np.float32)
    betas[:, 0] = beta1
    betas[:, 1] = beta2
    return x_bf, auxb, b3row, w3row, betas, beta1, beta2


import os

VERSION = os.environ.get("KERNEL_V", "tp")
N_PE = int(os.environ.get("KERNEL_NPE", "6"))
N_BLOCKS = int(os.environ.get("KERNEL_NBLK", "4"))


def _get_nc_and_inmaps(x, kernels, bias):
    if VERSION == "tp":
        x_bf, prmb, prmf, beta1, beta2 = _prep_tp_inputs(x, kernels, bias)
        key = (VERSION, N_BLOCKS, beta1, beta2)
        if key not in _compiled:
            _compiled[key] = _build_tp(beta1, beta2, n_blocks=N_BLOCKS)
        nc = _compiled[key]
        in_maps = [
            {
                "xt": np.ascontiguousarray(
                    x_bf[c * B_CORE : (c + 1) * B_CORE, :].T
                ),
                "prmb": prmb,
                "prmf": prmf,
            }
            for c in range(N_CORES)
        ]
        return nc, in_maps
    if VERSION == "b16":
        x_bf, auxb, b3row, w3row, betas, beta1, beta2 = _prep_b16_inputs(
            x, kernels, bias
        )
        key = (VERSION, N_PE, beta1, beta2)
        if key not in _compiled:
            _compiled[key] = _build_b16(beta1, beta2, n_pe=N_PE)
        nc = _compiled[key]
        in_maps = [
            {
                "x": x_bf[c * B_CORE : (c + 1) * B_CORE],
                "auxb": auxb,
                "b3row": b3row,
                "w3row": w3row,
                "betas": betas,
            }
            for c in range(N_CORES)
        ]
        return nc, in_maps
    x, aux, b3row, beta1, beta2 = _prep_pe_inputs(x, kernels, bias)
    key = (VERSION, N_PE, beta1, beta2)
    if key not in _compiled:
        if VERSION == "h2":
            _compiled[key] = _build_h2(beta1, beta2, n_pe=N_PE)
        elif VERSION == "hybrid":
            _compiled[key] = _build_hybrid(beta1, beta2, n_pe=N_PE)
        elif VERSION == "pe":
            _compiled[key] = _build_pe(beta1, beta2)
        else:
            _compiled[key] = _build(beta1, beta2)
    nc = _compiled[key]
    if VERSION == "h2":
        W = np.ascontiguousarray(kernels[:, :, 0], dtype=np.float32)
        in_maps = [
            {
                "x": x[c * B_CORE : (c + 1) * B_CORE],
                "aux": aux,
                "b3row": b3row,
                "w3row": W,
            }
            for c in range(N_CORES)
        ]
    elif VERSION == "hybrid":
        W = np.ascontiguousarray(kernels[:, :, 0], dtype=np.float32)
        wrep = np.broadcast_to(
            np.concatenate([W[0], W[1], W[2]]), (P, 3 * F)
        ).copy()
        in_maps = [
            {
                "x": x[c * B_CORE : (c + 1) * B_CORE],
                "aux": aux,
                "b3row": b3row,
                "wrep": wrep,
            }
            for c in range(N_CORES)
        ]
    elif VERSION == "pe":
        in_maps = [
            {"x": x[c * B_CORE : (c + 1) * B_CORE], "aux": aux, "b3row": b3row}
            for c in range(N_CORES)
        ]
    else:
        W = np.ascontiguousarray(kernels[:, :, 0], dtype=np.float32)
        Bb = np.ascontiguousarray(bias[:, :, 0], dtype=np.float32)
        b3 = Bb[0] + Bb[1] + Bb[2]
        wb = np.concatenate([W[0], W[1], W[2], b3]).astype(np.float32)
        wb = np.broadcast_to(wb, (P, 4 * F)).copy()
        in_maps = [
            {"x": x[c * B_CORE : (c + 1) * B_CORE], "wb": wb}
            for c in range(N_CORES)
        ]
    return nc, in_maps


def kernel(x: np.ndarray, kernels: np.ndarray, bias: np.ndarray) -> np.ndarray:
    nc, in_maps = _get_nc_and_inmaps(x, kernels, bias)
    last_err = None
    for _attempt in range(3):
        try:
            res = run_bass_kernel_spmd(nc, in_maps, core_ids=list(range(N_CORES)))
            break
        except Exception as e:
            last_err = e
    else:
        raise last_err
    if VERSION == "tp":
        outt = np.concatenate(
            [np.asarray(res.results[c]["outt"]) for c in range(N_CORES)], axis=1
        )
        return outt.T.astype(np.float32)
    out = np.concatenate(
        [np.asarray(res.results[c]["out"]) for c in range(N_CORES)], axis=0
    )
    return out.astype(np.float32)


def timed_run(x, kernels, bias):
    nc, in_maps = _get_nc_and_inmaps(x, kernels, bias)
    res = run_bass_kernel_spmd(
        nc, in_maps, core_ids=list(range(N_CORES)), trace=True
    )
    print(
        "exec_time_ns:", res.exec_time_ns,
        "mean:", res.mean_exec_time_ns,
        "max core:", res.max_exec_time_core_id,
    )
    if res.instructions_and_trace:
        print("trace:", res.instructions_and_trace[1])
    return res.exec_time_ns


if __name__ == "__main__":
    rng = np.random.default_rng(0)
    x = rng.standard_normal((B_FULL, F), dtype=np.float32)
    k = rng.standard_normal((3, F, 1), dtype=np.float32) * 0.07
    b = rng.standard_normal((3, F, 1), dtype=np.float32) * 0.07
    out = kernel(x=x, kernels=k, bias=b)
    print("out", out.shape, out.dtype)



# revision 45
# speedup vs baseline: 1.0679x; 1.0679x over previous
import sys

sys.path.insert(0, "/opt/trn_rl_repo")

import numpy as np

import concourse.bass as bass
import concourse.tile as tile
from concourse import bacc, mybir
from concourse.bass_utils import run_bass_kernel_spmd

N_CORES = 8
B_FULL, F = 16384, 1024
B_CORE = B_FULL // N_CORES
P = 128
N_TILES = B_CORE // P

_compiled = {}


def _build(beta1: float, beta2: float, n_tiles: int = N_TILES):
    b_core = n_tiles * P
    nc = bacc.Bacc(
        "TRN2",
        target_bir_lowering=False,
        debug=False,
        enable_asserts=False,
        num_devices=N_CORES,
    )
    f32 = mybir.dt.float32
    x_d = nc.dram_tensor("x", [b_core, F], f32, kind="ExternalInput").ap()
    wb_d = nc.dram_tensor("wb", [P, 4 * F], f32, kind="ExternalInput").ap()
    out_d = nc.dram_tensor("out", [b_core, F], f32, kind="ExternalOutput").ap()

    x_r = x_d.rearrange("(n p) f -> n p f", p=P)
    out_r = out_d.rearrange("(n p) f -> n p f", p=P)

    AT = mybir.AluOpType

    with tile.TileContext(nc) as tc:
        with (
            tc.tile_pool(name="params", bufs=1) as params,
            tc.tile_pool(name="xp", bufs=4) as xp,
            tc.tile_pool(name="junk", bufs=2) as junkp,
            tc.tile_pool(name="small", bufs=4) as smallp,
            tc.tile_pool(name="outp", bufs=4) as outp,
        ):
            wb = params.tile([P, 4 * F], f32)
            nc.sync.dma_start(wb[:], wb_d[:])
            w = [wb[:, l * F : (l + 1) * F] for l in range(3)]
            b3 = wb[:, 3 * F : 4 * F]

            for i in range(n_tiles):
                x_t = xp.tile([P, F], f32)
                nc.sync.dma_start(x_t[:], x_r[i])

                a = smallp.tile([P, 3], f32, tag="a")
                junk = junkp.tile([P, F], f32)
                for l in range(3):
                    nc.vector.scalar_tensor_tensor(
                        out=junk[:],
                        in0=x_t[:],
                        scalar=1.0,
                        in1=w[l],
                        op0=AT.mult,
                        op1=AT.mult,
                        accum_out=a[:, l : l + 1],
                    )

                c1 = smallp.tile([P, 1], f32, tag="c1")
                nc.vector.tensor_scalar_add(c1[:], a[:, 0:1], 1.0)
                s1 = smallp.tile([P, 1], f32, tag="s1")
                nc.vector.tensor_scalar(
                    s1[:], a[:, 1:2], c1[:, 0:1], beta1, AT.mult, AT.add
                )
                c2 = smallp.tile([P, 1], f32, tag="c2")
                nc.vector.tensor_add(c2[:], c1[:], s1[:])
                s2 = smallp.tile([P, 1], f32, tag="s2")
                nc.vector.tensor_scalar(
                    s2[:], a[:, 2:3], c2[:, 0:1], beta2, AT.mult, AT.add
                )
                c3 = smallp.tile([P, 1], f32, tag="c3")
                nc.vector.tensor_add(c3[:], c2[:], s2[:])

                o_t = outp.tile([P, F], f32)
                nc.vector.scalar_tensor_tensor(
                    out=o_t[:], in0=x_t[:], scalar=c3[:, 0:1], in1=b3,
                    op0=AT.mult, op1=AT.add,
                )
                nc.scalar.dma_start(out_r[i], o_t[:])

    nc.compile()
    return nc


def _build_pe(beta1: float, beta2: float, n_tiles: int = N_TILES):
    b_core = n_tiles * P
    nc = bacc.Bacc(
        "TRN2",
        target_bir_lowering=False,
        debug=False,
        enable_asserts=False,
        num_devices=N_CORES,
    )
    f32 = mybir.dt.float32
    x_d = nc.dram_tensor("x", [b_core, F], f32, kind="ExternalInput").ap()
    aux_d = nc.dram_tensor("aux", [P, 162], f32, kind="ExternalInput").ap()
    b3_d = nc.dram_tensor("b3row", [1, F], f32, kind="ExternalInput").ap()
    out_d = nc.dram_tensor("out", [b_core, F], f32, kind="ExternalOutput").ap()

    x_r = x_d.rearrange("(n p) f -> n p f", p=P)
    out_r = out_d.rearrange("(n p) f -> n p f", p=P)

    AT = mybir.AluOpType
    AF = mybir.ActivationFunctionType

    with tile.TileContext(nc) as tc:
        with (
            tc.tile_pool(name="params", bufs=1) as params,
            tc.tile_pool(name="xp", bufs=4) as xp,
            tc.tile_pool(name="sbT", bufs=3) as sbTp,
            tc.tile_pool(name="psT", bufs=2, space="PSUM") as psTp,
            tc.tile_pool(name="psA", bufs=2, space="PSUM") as psAp,
            tc.tile_pool(name="small", bufs=4) as smallp,
            tc.tile_pool(name="outp", bufs=4) as outp,
        ):
            aux = params.tile([P, 162], f32)
            nc.sync.dma_start(aux[:], aux_d[:])
            ident = aux[:, 0:128]
            wsb = aux[:, 128:160]
            betas = aux[:, 160:162]

            b3s = params.tile([1, F], f32, tag="b3s")
            nc.sync.dma_start(b3s[:], b3_d[:])
            ones = params.tile([1, P], f32, tag="ones")
            nc.vector.memset(ones[:], 1.0)
            b3rep = params.tile([P, F], f32, tag="b3rep")
            for j in range(2):
                pb = psAp.tile([P, 512], f32, tag="pb")
                nc.tensor.matmul(
                    pb[:], ones[0:1, :], b3s[0:1, j * 512 : (j + 1) * 512],
                    start=True, stop=True,
                )
                nc.scalar.copy(b3rep[:, j * 512 : (j + 1) * 512], pb[:])

            for i in range(n_tiles):
                x_t = xp.tile([P, F], f32)
                nc.sync.dma_start(x_t[:], x_r[i])

                psT = psTp.tile([P, F], f32)
                for k in range(8):
                    nc.tensor.transpose(
                        psT[:, k * P : (k + 1) * P],
                        x_t[:, k * P : (k + 1) * P],
                        ident,
                    )
                sbT = sbTp.tile([P, F], f32)
                nc.scalar.copy(sbT[:], psT[:])

                psA = psAp.tile([P, 3], f32, tag="a")
                for k in range(8):
                    nc.tensor.matmul(
                        psA[:],
                        sbT[:, k * P : (k + 1) * P],
                        wsb[:, 4 * k : 4 * k + 3],
                        start=(k == 0),
                        stop=(k == 7),
                    )

                c1 = smallp.tile([P, 1], f32, tag="c1")
                nc.scalar.activation(c1[:], psA[:, 0:1], AF.Identity, bias=1.0)
                s1 = smallp.tile([P, 1], f32, tag="s1")
                nc.scalar.activation(
                    s1[:], psA[:, 1:2], AF.Identity,
                    bias=betas[:, 0:1], scale=c1[:, 0:1],
                )
                c2 = smallp.tile([P, 1], f32, tag="c2")
                nc.scalar.activation(
                    c2[:], c1[:], AF.Identity, bias=s1[:, 0:1]
                )
                s2 = smallp.tile([P, 1], f32, tag="s2")
                nc.scalar.activation(
                    s2[:], psA[:, 2:3], AF.Identity,
                    bias=betas[:, 1:2], scale=c2[:, 0:1],
                )
                c3 = smallp.tile([P, 1], f32, tag="c3")
                nc.scalar.activation(
                    c3[:], c2[:], AF.Identity, bias=s2[:, 0:1]
                )

                o_t = outp.tile([P, F], f32)
                nc.vector.scalar_tensor_tensor(
                    out=o_t[:], in0=x_t[:], scalar=c3[:, 0:1], in1=b3rep[:],
                    op0=AT.mult, op1=AT.add,
                )
                nc.scalar.dma_start(out_r[i], o_t[:])

    nc.compile()
    return nc


def _build_hybrid(beta1: float, beta2: float, n_tiles: int = N_TILES, n_pe: int = 6):
    b_core = n_tiles * P
    nc = bacc.Bacc(
        "TRN2",
        target_bir_lowering=False,
        debug=False,
        enable_asserts=False,
        num_devices=N_CORES,
    )
    f32 = mybir.dt.float32
    x_d = nc.dram_tensor("x", [b_core, F], f32, kind="ExternalInput").ap()
    aux_d = nc.dram_tensor("aux", [P, 162], f32, kind="ExternalInput").ap()
    b3_d = nc.dram_tensor("b3row", [1, F], f32, kind="ExternalInput").ap()
    wrep_d = nc.dram_tensor("wrep", [P, 3 * F], f32, kind="ExternalInput").ap()
    out_d = nc.dram_tensor("out", [b_core, F], f32, kind="ExternalOutput").ap()

    x_r = x_d.rearrange("(n p) f -> n p f", p=P)
    out_r = out_d.rearrange("(n p) f -> n p f", p=P)

    AT = mybir.AluOpType
    AF = mybir.ActivationFunctionType

    pe_set = {i for i in range(n_tiles) if (i + 1) * n_pe // n_tiles > i * n_pe // n_tiles}

    with tile.TileContext(nc) as tc:
        with (
            tc.tile_pool(name="params", bufs=1) as params,
            tc.tile_pool(name="xp", bufs=4) as xp,
            tc.tile_pool(name="junk", bufs=2) as junkp,
            tc.tile_pool(name="sbT", bufs=3) as sbTp,
            tc.tile_pool(name="psT", bufs=2, space="PSUM") as psTp,
            tc.tile_pool(name="psA", bufs=2, space="PSUM") as psAp,
            tc.tile_pool(name="small", bufs=4) as smallp,
            tc.tile_pool(name="outp", bufs=4) as outp,
        ):
            aux = params.tile([P, 162], f32)
            nc.sync.dma_start(aux[:], aux_d[:])
            ident = aux[:, 0:128]
            wsb = aux[:, 128:160]
            betas = aux[:, 160:162]

            wrep = params.tile([P, 3 * F], f32, tag="wrep")
            nc.sync.dma_start(wrep[:], wrep_d[:])
            wv = [wrep[:, l * F : (l + 1) * F] for l in range(3)]

            b3s = params.tile([1, F], f32, tag="b3s")
            nc.sync.dma_start(b3s[:], b3_d[:])
            ones = params.tile([1, P], f32, tag="ones")
            nc.vector.memset(ones[:], 1.0)
            b3rep = params.tile([P, F], f32, tag="b3rep")
            for j in range(2):
                pb = psAp.tile([P, 512], f32, tag="pb")
                nc.tensor.matmul(
                    pb[:], ones[0:1, :], b3s[0:1, j * 512 : (j + 1) * 512],
                    start=True, stop=True,
                )
                nc.scalar.copy(b3rep[:, j * 512 : (j + 1) * 512], pb[:])

            def recurrence(a_ap):
                c1 = smallp.tile([P, 1], f32, tag="c1")
                nc.scalar.activation(c1[:], a_ap[:, 0:1], AF.Identity, bias=1.0)
                s1 = smallp.tile([P, 1], f32, tag="s1")
                nc.scalar.activation(
                    s1[:], a_ap[:, 1:2], AF.Identity,
                    bias=betas[:, 0:1], scale=c1[:, 0:1],
                )
                c2 = smallp.tile([P, 1], f32, tag="c2")
                nc.scalar.activation(c2[:], c1[:], AF.Identity, bias=s1[:, 0:1])
                s2 = smallp.tile([P, 1], f32, tag="s2")
                nc.scalar.activation(
                    s2[:], a_ap[:, 2:3], AF.Identity,
                    bias=betas[:, 1:2], scale=c2[:, 0:1],
                )
                c3 = smallp.tile([P, 1], f32, tag="c3")
                nc.scalar.activation(c3[:], c2[:], AF.Identity, bias=s2[:, 0:1])
                return c3

            for i in range(n_tiles):
                x_t = xp.tile([P, F], f32)
                nc.sync.dma_start(x_t[:], x_r[i])

                if i in pe_set:
                    psT = psTp.tile([P, F], f32)
                    for k in range(8):
                        nc.tensor.transpose(
                            psT[:, k * P : (k + 1) * P],
                            x_t[:, k * P : (k + 1) * P],
                            ident,
                        )
                    sbT = sbTp.tile([P, F], f32)
                    nc.scalar.copy(sbT[:], psT[:])
                    psA = psAp.tile([P, 3], f32, tag="a")
                    for k in range(8):
                        nc.tensor.matmul(
                            psA[:],
                            sbT[:, k * P : (k + 1) * P],
                            wsb[:, 4 * k : 4 * k + 3],
                            start=(k == 0),
                            stop=(k == 7),
                        )
                    c3 = recurrence(psA)
                else:
                    a = smallp.tile([P, 3], f32, tag="adve")
                    junk = junkp.tile([P, F], f32)
                    for l in range(3):
                        nc.vector.scalar_tensor_tensor(
                            out=junk[:], in0=x_t[:], scalar=1.0, in1=wv[l],
                            op0=AT.mult, op1=AT.mult,
                            accum_out=a[:, l : l + 1],
                        )
                    c3 = recurrence(a)

                o_t = outp.tile([P, F], f32)
                nc.vector.scalar_tensor_tensor(
                    out=o_t[:], in0=x_t[:], scalar=c3[:, 0:1], in1=b3rep[:],
                    op0=AT.mult, op1=AT.add,
                )
                nc.scalar.dma_start(out_r[i], o_t[:])

    nc.compile()
    return nc


def _build_h2(beta1: float, beta2: float, n_tiles: int = N_TILES, n_pe: int = 12):
    b_core = n_tiles * P
    assert n_tiles % 4 == 0
    nc = bacc.Bacc(
        "TRN2",
        target_bir_lowering=False,
        debug=False,
        enable_asserts=False,
        num_devices=N_CORES,
    )
    f32 = mybir.dt.float32
    f32r = mybir.dt.float32r
    x_d = nc.dram_tensor("x", [b_core, F], f32, kind="ExternalInput").ap()
    aux_d = nc.dram_tensor("aux", [P, 162], f32, kind="ExternalInput").ap()
    b3_d = nc.dram_tensor("b3row", [1, F], f32, kind="ExternalInput").ap()
    w3_d = nc.dram_tensor("w3row", [3, F], f32, kind="ExternalInput").ap()
    out_d = nc.dram_tensor("out", [b_core, F], f32, kind="ExternalOutput").ap()

    x_r = x_d.rearrange("(n p) f -> n p f", p=P)
    out_r = out_d.rearrange("(n p) f -> n p f", p=P)

    AT = mybir.AluOpType

    pe_set = {i for i in range(n_tiles) if (i + 1) * n_pe // n_tiles > i * n_pe // n_tiles}

    with tile.TileContext(nc) as tc:
        with (
            tc.tile_pool(name="params", bufs=1) as params,
            tc.tile_pool(name="xp", bufs=16) as xp,
            tc.tile_pool(name="junk", bufs=3) as junkp,
            tc.tile_pool(name="sbT", bufs=3) as sbTp,
            tc.tile_pool(name="psT", bufs=2, space="PSUM") as psTp,
            tc.tile_pool(name="psA", bufs=2, space="PSUM") as psAp,
            tc.tile_pool(name="psB", bufs=2, space="PSUM") as psBp,
            tc.tile_pool(name="small", bufs=2) as smallp,
            tc.tile_pool(name="outp", bufs=10) as outp,
        ):
            aux = params.tile([P, 162], f32)
            nc.sync.dma_start(aux[:], aux_d[:])
            ident = aux[:, 0:128]
            wsb = aux[:, 128:160]

            b3s = params.tile([1, F], f32, tag="b3s")
            nc.sync.dma_start(b3s[:], b3_d[:])
            w3s = []
            for l in range(3):
                t = params.tile([1, F], f32, tag=f"w3s{l}")
                nc.sync.dma_start(t[:], w3_d[l : l + 1, :])
                w3s.append(t)
            ones = params.tile([1, P], f32, tag="ones")
            nc.vector.memset(ones[:], 1.0)
            wv = []
            for l in range(3):
                wrep_l = params.tile([P, F], f32, tag=f"w{l}rep", name=f"w{l}rep")
                wv.append(wrep_l[:])
            b3rep = params.tile([P, F], f32, tag="b3rep")
            bcasts = [(wv[l], w3s[l][0:1, :]) for l in range(3)]
            bcasts.append((b3rep[:], b3s[0:1, :]))
            for dst, src in bcasts:
                for j in range(2):
                    pb = psBp.tile([P, 512], f32, tag="pb")
                    nc.tensor.matmul(
                        pb[:], ones[0:1, :], src[:, j * 512 : (j + 1) * 512],
                        start=True, stop=True,
                    )
                    nc.scalar.copy(dst[:, j * 512 : (j + 1) * 512], pb[:])

            def dve_recurrence(a_grp, c3g, width):
                av = a_grp[:, 0 : 3 * width].rearrange("p (j l) -> p j l", l=3)
                a0, a1, a2 = av[:, :, 0], av[:, :, 1], av[:, :, 2]
                c1 = smallp.tile([P, 4], f32, tag="c1")
                nc.vector.tensor_scalar_add(c1[:, 0:width], a0, 1.0)
                s1p = smallp.tile([P, 4], f32, tag="s1p")
                nc.vector.scalar_tensor_tensor(
                    out=s1p[:, 0:width], in0=a1, scalar=1.0, in1=c1[:, 0:width],
                    op0=AT.mult, op1=AT.mult,
                )
                c2 = smallp.tile([P, 4], f32, tag="c2")
                nc.vector.scalar_tensor_tensor(
                    out=c2[:, 0:width], in0=c1[:, 0:width], scalar=beta1,
                    in1=s1p[:, 0:width], op0=AT.add, op1=AT.add,
                )
                s2p = smallp.tile([P, 4], f32, tag="s2p")
                nc.vector.scalar_tensor_tensor(
                    out=s2p[:, 0:width], in0=a2, scalar=1.0, in1=c2[:, 0:width],
                    op0=AT.mult, op1=AT.mult,
                )
                nc.vector.scalar_tensor_tensor(
                    out=c3g[:, 0:width], in0=c2[:, 0:width], scalar=beta2,
                    in1=s2p[:, 0:width], op0=AT.add, op1=AT.add,
                )

            def act_recurrence(psA, betas):
                AF = mybir.ActivationFunctionType
                c1 = smallp.tile([P, 1], f32, tag="pc1")
                nc.scalar.activation(c1[:], psA[:, 0:1], AF.Identity, bias=1.0)
                s1 = smallp.tile([P, 1], f32, tag="ps1")
                nc.scalar.activation(
                    s1[:], psA[:, 1:2], AF.Identity,
                    bias=betas[:, 0:1], scale=c1[:, 0:1],
                )
                c2 = smallp.tile([P, 1], f32, tag="pc2")
                nc.scalar.activation(c2[:], c1[:], AF.Identity, bias=s1[:, 0:1])
                s2 = smallp.tile([P, 1], f32, tag="ps2")
                nc.scalar.activation(
                    s2[:], psA[:, 2:3], AF.Identity,
                    bias=betas[:, 1:2], scale=c2[:, 0:1],
                )
                c3 = smallp.tile([P, 1], f32, tag="pc3")
                nc.scalar.activation(c3[:], c2[:], AF.Identity, bias=s2[:, 0:1])
                return c3

            betas = aux[:, 160:162]
            dve_grp = []
            a_grp = None
            c3g = None

            def flush_dve_group():
                nonlocal dve_grp, a_grp, c3g
                if not dve_grp:
                    return
                dve_recurrence(a_grp, c3g, len(dve_grp))
                for j, (i, x_t) in enumerate(dve_grp):
                    o_t = outp.tile([P, F], f32)
                    nc.vector.scalar_tensor_tensor(
                        out=o_t[:], in0=x_t[:], scalar=c3g[:, j : j + 1],
                        in1=b3rep[:], op0=AT.mult, op1=AT.add,
                    )
                    nc.scalar.dma_start(out_r[i], o_t[:])
                dve_grp = []
                a_grp = None
                c3g = None

            for i in range(n_tiles):
                x_t = xp.tile([P, F], f32)
                nc.sync.dma_start(x_t[:], x_r[i])

                if i in pe_set:
                    psT = psTp.tile([P, F], f32)
                    for k in range(8):
                        nc.tensor.transpose(
                            psT[:, k * P : (k + 1) * P],
                            x_t[:, k * P : (k + 1) * P],
                            ident,
                        )
                    sbT = sbTp.tile([P, F], f32)
                    nc.scalar.copy(sbT[:], psT[:])
                    psA = psAp.tile([P, 3], f32, tag="a")
                    for k in range(8):
                        nc.tensor.matmul(
                            psA[:],
                            sbT[:, k * P : (k + 1) * P],
                            wsb[:, 4 * k : 4 * k + 3],
                            start=(k == 0),
                            stop=(k == 7),
                        )
                    c3 = act_recurrence(psA, betas)
                    o_t = outp.tile([P, F], f32)
                    nc.vector.scalar_tensor_tensor(
                        out=o_t[:], in0=x_t[:], scalar=c3[:, 0:1],
                        in1=b3rep[:], op0=AT.mult, op1=AT.add,
                    )
                    nc.scalar.dma_start(out_r[i], o_t[:])
                else:
                    if not dve_grp:
                        a_grp = smallp.tile([P, 12], f32, tag="ag")
                        c3g = smallp.tile([P, 4], f32, tag="c3g")
                    j = len(dve_grp)
                    junk = junkp.tile([P, F], f32)
                    for l in range(3):
                        nc.vector.scalar_tensor_tensor(
                            out=junk[:], in0=x_t[:], scalar=1.0, in1=wv[l],
                            op0=AT.mult, op1=AT.mult,
                            accum_out=a_grp[:, 3 * j + l : 3 * j + l + 1],
                        )
                    dve_grp.append((i, x_t))
                    if len(dve_grp) == 4:
                        flush_dve_group()
            flush_dve_group()

    nc.compile()
    return nc


def _build_b16(beta1: float, beta2: float, n_tiles: int = N_TILES, n_pe: int = 6):
    b_core = n_tiles * P
    assert n_tiles % 4 == 0
    nc = bacc.Bacc(
        "TRN2",
        target_bir_lowering=False,
        debug=False,
        enable_asserts=False,
        num_devices=N_CORES,
    )
    f32 = mybir.dt.float32
    bf16 = mybir.dt.bfloat16
    x_d = nc.dram_tensor("x", [b_core, F], bf16, kind="ExternalInput").ap()
    aux_d = nc.dram_tensor("auxb", [P, 160], bf16, kind="ExternalInput").ap()
    b3_d = nc.dram_tensor("b3row", [1, F], bf16, kind="ExternalInput").ap()
    w3_d = nc.dram_tensor("w3row", [3, F], bf16, kind="ExternalInput").ap()
    bt_d = nc.dram_tensor("betas", [P, 2], f32, kind="ExternalInput").ap()
    out_d = nc.dram_tensor("out", [b_core, F], bf16, kind="ExternalOutput").ap()

    x_r2 = x_d.rearrange("(n two p) f -> n p two f", two=2, p=P)
    out_r2 = out_d.rearrange("(n two p) f -> n p two f", two=2, p=P)

    AT = mybir.AluOpType
    AF = mybir.ActivationFunctionType

    pe_set = {i for i in range(n_tiles) if (i + 1) * n_pe // n_tiles > i * n_pe // n_tiles}

    with tile.TileContext(nc) as tc:
        with (
            tc.tile_pool(name="params", bufs=1) as params,
            tc.tile_pool(name="xp", bufs=8) as xp,
            tc.tile_pool(name="junk", bufs=3) as junkp,
            tc.tile_pool(name="sbT", bufs=3) as sbTp,
            tc.tile_pool(name="psT", bufs=2, space="PSUM") as psTp,
            tc.tile_pool(name="psA", bufs=2, space="PSUM") as psAp,
            tc.tile_pool(name="psB", bufs=2, space="PSUM") as psBp,
            tc.tile_pool(name="small", bufs=2) as smallp,
            tc.tile_pool(name="outp", bufs=6) as outp,
        ):
            aux = params.tile([P, 160], bf16)
            nc.sync.dma_start(aux[:], aux_d[:])
            ident = aux[:, 0:128]
            wsb = aux[:, 128:160]
            betas = params.tile([P, 2], f32, tag="betas")
            nc.sync.dma_start(betas[:], bt_d[:])

            b3s = params.tile([1, F], bf16, tag="b3s")
            nc.sync.dma_start(b3s[:], b3_d[:])
            w3s = []
            for l in range(3):
                t = params.tile([1, F], bf16, tag=f"w3s{l}")
                nc.sync.dma_start(t[:], w3_d[l : l + 1, :])
                w3s.append(t)
            ones = params.tile([1, P], bf16, tag="ones")
            nc.vector.memset(ones[:], 1.0)
            wv = []
            for l in range(3):
                wrep_l = params.tile([P, F], bf16, tag=f"w{l}rep", name=f"w{l}rep")
                wv.append(wrep_l[:])
            b3rep = params.tile([P, F], bf16, tag="b3rep")
            bcasts = [(wv[l], w3s[l][0:1, :]) for l in range(3)]
            bcasts.append((b3rep[:], b3s[0:1, :]))
            for dst, src in bcasts:
                for j in range(2):
                    pb = psBp.tile([P, 512], f32, tag="pb")
                    nc.tensor.matmul(
                        pb[:], ones[0:1, :], src[:, j * 512 : (j + 1) * 512],
                        start=True, stop=True,
                    )
                    nc.scalar.copy(dst[:, j * 512 : (j + 1) * 512], pb[:])

            def dve_recurrence(a_grp, c3g, width):
                av = a_grp[:, 0 : 3 * width].rearrange("p (j l) -> p j l", l=3)
                a0, a1, a2 = av[:, :, 0], av[:, :, 1], av[:, :, 2]
                c1 = smallp.tile([P, 4], f32, tag="c1")
                nc.vector.tensor_scalar_add(c1[:, 0:width], a0, 1.0)
                s1p = smallp.tile([P, 4], f32, tag="s1p")
                nc.vector.scalar_tensor_tensor(
                    out=s1p[:, 0:width], in0=a1, scalar=1.0, in1=c1[:, 0:width],
                    op0=AT.mult, op1=AT.mult,
                )
                c2 = smallp.tile([P, 4], f32, tag="c2")
                nc.vector.scalar_tensor_tensor(
                    out=c2[:, 0:width], in0=c1[:, 0:width], scalar=beta1,
                    in1=s1p[:, 0:width], op0=AT.add, op1=AT.add,
                )
                s2p = smallp.tile([P, 4], f32, tag="s2p")
                nc.vector.scalar_tensor_tensor(
                    out=s2p[:, 0:width], in0=a2, scalar=1.0, in1=c2[:, 0:width],
                    op0=AT.mult, op1=AT.mult,
                )
                nc.vector.scalar_tensor_tensor(
                    out=c3g[:, 0:width], in0=c2[:, 0:width], scalar=beta2,
                    in1=s2p[:, 0:width], op0=AT.add, op1=AT.add,
                )

            def act_recurrence(psA):
                c1 = smallp.tile([P, 1], f32, tag="pc1")
                nc.scalar.activation(c1[:], psA[:, 0:1], AF.Identity, bias=1.0)
                s1 = smallp.tile([P, 1], f32, tag="ps1")
                nc.scalar.activation(
                    s1[:], psA[:, 1:2], AF.Identity,
                    bias=betas[:, 0:1], scale=c1[:, 0:1],
                )
                c2 = smallp.tile([P, 1], f32, tag="pc2")
                nc.scalar.activation(c2[:], c1[:], AF.Identity, bias=s1[:, 0:1])
                s2 = smallp.tile([P, 1], f32, tag="ps2")
                nc.scalar.activation(
                    s2[:], psA[:, 2:3], AF.Identity,
                    bias=betas[:, 1:2], scale=c2[:, 0:1],
                )
                c3 = smallp.tile([P, 1], f32, tag="pc3")
                nc.scalar.activation(c3[:], c2[:], AF.Identity, bias=s2[:, 0:1])
                return c3

            xpairs = []
            for j in range(n_tiles // 2):
                x2 = xp.tile([P, 2 * F], bf16, tag="x2")
                nc.sync.dma_start(
                    x2[:].rearrange("p (two f) -> p two f", two=2), x_r2[j]
                )
                xpairs.append(x2)

            def x_tile(i):
                return xpairs[i // 2][:, (i % 2) * F : (i % 2 + 1) * F]

            opairs = {}

            def emit_out(i, c3_ap):
                j, h = i // 2, i % 2
                if j not in opairs:
                    opairs[j] = outp.tile([P, 2 * F], bf16, tag="o2", name="o2")
                o2 = opairs[j]
                nc.vector.scalar_tensor_tensor(
                    out=o2[:, h * F : (h + 1) * F], in0=x_tile(i), scalar=c3_ap,
                    in1=b3rep[:], op0=AT.mult, op1=AT.add,
                )
                done = opairs.get(("done", j), 0) + 1
                opairs[("done", j)] = done
                if done == 2:
                    nc.scalar.dma_start(
                        out_r2[j], o2[:].rearrange("p (two f) -> p two f", two=2)
                    )

            dve_grp = []
            a_grp = None
            c3g = None

            def flush_dve_group():
                nonlocal dve_grp, a_grp, c3g
                if not dve_grp:
                    return
                dve_recurrence(a_grp, c3g, len(dve_grp))
                for j, i in enumerate(dve_grp):
                    emit_out(i, c3g[:, j : j + 1])
                dve_grp = []
                a_grp = None
                c3g = None

            for i in range(n_tiles):
                x_t = x_tile(i)
                if i in pe_set:
                    psT = psTp.tile([P, F], bf16)
                    for k in range(8):
                        nc.tensor.transpose(
                            psT[:, k * P : (k + 1) * P],
                            x_t[:, k * P : (k + 1) * P],
                            ident,
                        )
                    sbT = sbTp.tile([P, F], bf16)
                    nc.scalar.copy(sbT[:], psT[:])
                    psA = psAp.tile([P, 3], f32, tag="a")
                    for k in range(8):
                        nc.tensor.matmul(
                            psA[:],
                            sbT[:, k * P : (k + 1) * P],
                            wsb[:, 4 * k : 4 * k + 3],
                            start=(k == 0),
                            stop=(k == 7),
                        )
                    c3 = act_recurrence(psA)
                    emit_out(i, c3[:, 0:1])
                else:
                    if not dve_grp:
                        a_grp = smallp.tile([P, 12], f32, tag="ag")
                        c3g = smallp.tile([P, 4], f32, tag="c3g")
                    j = len(dve_grp)
                    junk = junkp.tile([P, F], bf16)
                    for l in range(3):
                        nc.vector.scalar_tensor_tensor(
                            out=junk[:], in0=x_t[:], scalar=1.0, in1=wv[l],
                            op0=AT.mult, op1=AT.mult,
                            accum_out=a_grp[:, 3 * j + l : 3 * j + l + 1],
                        )
                    dve_grp.append(i)
                    if len(dve_grp) == 4:
                        flush_dve_group()
            flush_dve_group()

    nc.compile()
    return nc


def _prep_pe_inputs(x, kernels, bias):
    x = np.ascontiguousarray(x, dtype=np.float32)
    W = np.ascontiguousarray(kernels[:, :, 0], dtype=np.float32)
    Bb = np.ascontiguousarray(bias[:, :, 0], dtype=np.float32)
    beta1 = float(Bb[0] @ W[1])
    beta2 = float((Bb[0] + Bb[1]) @ W[2])
    b3 = (Bb[0] + Bb[1] + Bb[2]).astype(np.float32)

    aux = np.zeros((P, 162), dtype=np.float32)
    aux[:, 0:128] = np.eye(P, dtype=np.float32)
    wsb4 = np.zeros((P, 8, 4), dtype=np.float32)
    wsb4[:, :, 0:3] = W.T.reshape(8, P, 3).transpose(1, 0, 2)
    aux[:, 128:160] = wsb4.reshape(P, 32)
    aux[:, 160] = beta1
    aux[:, 161] = beta2
    b3row = b3.reshape(1, F)
    return x, aux, b3row, beta1, beta2


def _build_tp(beta1: float, beta2: float, n_blocks: int = 5):
    if n_blocks == 5:
        BLKS = [128, 384, 512, 512, 512]
    else:
        BLKS = [B_CORE // n_blocks] * n_blocks
    assert sum(BLKS) == B_CORE and all(b % P == 0 for b in BLKS)
    OFFS = [sum(BLKS[:i]) for i in range(len(BLKS))]
    NCH = F // P
    nc = bacc.Bacc(
        "TRN2",
        target_bir_lowering=False,
        debug=False,
        enable_asserts=False,
        num_devices=N_CORES,
    )
    f32 = mybir.dt.float32
    bf16 = mybir.dt.bfloat16
    xt_d = nc.dram_tensor("xt", [F, B_CORE], bf16, kind="ExternalInput").ap()
    prmb_d = nc.dram_tensor("prmb", [P, 32], bf16, kind="ExternalInput").ap()
    prmf_d = nc.dram_tensor("prmf", [P, 140], f32, kind="ExternalInput").ap()
    outt_d = nc.dram_tensor("outt", [F, B_CORE], bf16, kind="ExternalOutput").ap()

    def io_view(dram, g):
        return dram[:, OFFS[g] : OFFS[g] + BLKS[g]].rearrange(
            "(h k p) b -> h p k b", h=2, k=NCH // 2, p=P
        )

    AT = mybir.AluOpType
    AF = mybir.ActivationFunctionType

    with tile.TileContext(nc) as tc:
        with (
            tc.tile_pool(name="params", bufs=1) as params,
            tc.tile_pool(name="xp", bufs=4) as xp,
            tc.tile_pool(name="asb", bufs=3) as asbp,
            tc.tile_pool(name="smalls", bufs=3) as smallp,
            tc.tile_pool(name="c3rp", bufs=3) as c3rp,
            tc.tile_pool(name="tmp", bufs=3) as tmpp,
            tc.tile_pool(name="otp", bufs=4) as otp,
            tc.tile_pool(name="psA", bufs=2, space="PSUM") as psAp,
            tc.tile_pool(name="psAT", bufs=2, space="PSUM") as psATp,
            tc.tile_pool(name="psC", bufs=2, space="PSUM") as psCp,
            tc.tile_pool(name="psR", bufs=2, space="PSUM") as psRp,
        ):
            prmb = params.tile([P, 32], bf16)
            nc.sync.dma_start(prmb[:], prmb_d[:])
            wck = prmb[:, 0:32]
            prmf = params.tile([P, 140], f32, tag="prmf")
            nc.sync.dma_start(prmf[:], prmf_d[:])
            ident128 = prmf[:, 0:128]
            ident3 = prmf[0:3, 128:131]
            b3cols = prmf[:, 132:140]
            onesf = params.tile([1, P], f32, tag="onesf")
            nc.vector.memset(onesf[:], 1.0)

            BMAX = max(BLKS)
            b3rep = params.tile([P, NCH * BMAX], bf16, tag="b3rep")
            b3bc = (
                b3cols[:, 0:NCH]
                .rearrange("p (k o) -> p k o", o=1)
                .broadcast_to([P, NCH, BMAX])
            )
            nc.vector.tensor_copy(
                b3rep[:].rearrange("p (k b) -> p k b", k=NCH), b3bc
            )

            xtbs = {}
            psATs = {}
            c3blks = {}
            c3reps = {}
            tmps = {}

            def stage_load(g):
                B = BLKS[g]
                xtb = xp.tile([P, NCH * B], bf16, name="xtb")
                xv = xtb[:].rearrange("p (h k b) -> h p k b", h=2, b=B)
                src = io_view(xt_d, g)
                for h in range(2):
                    nc.sync.dma_start(xv[h], src[h])
                xtbs[g] = xtb

            def stage_dots(g):
                B = BLKS[g]
                xtb = xtbs[g]
                psA = psAp.tile([3, B], f32, tag="a", name="psA")
                for k in range(NCH):
                    nc.tensor.matmul(
                        psA[:],
                        wck[:, 4 * k : 4 * k + 3],
                        xtb[:, k * B : (k + 1) * B],
                        start=(k == 0),
                        stop=(k == NCH - 1),
                    )
                aSB = asbp.tile([3, B], f32, tag="asb", name="aSB")
                nc.scalar.copy(aSB[:], psA[:])
                T = B // P
                psAT = psATp.tile([P, 3 * T], f32, tag="at", name="psAT")
                for t in range(T):
                    nc.tensor.transpose(
                        psAT[:, 3 * t : 3 * t + 3],
                        aSB[:, t * P : (t + 1) * P],
                        ident3,
                    )
                psATs[g] = psAT

            def stage_rec(g):
                T = BLKS[g] // P
                psAT = psATs.pop(g)
                av = psAT[:].rearrange("p (t l) -> p t l", l=3)
                a0, a1, a2 = av[:, :, 0], av[:, :, 1], av[:, :, 2]
                c1 = smallp.tile([P, T], f32, tag="c1", name="c1")
                nc.vector.tensor_scalar_add(c1[:], a0, 1.0)
                s1p = smallp.tile([P, T], f32, tag="s1p", name="s1p")
                nc.vector.scalar_tensor_tensor(
                    out=s1p[:], in0=a1, scalar=1.0, in1=c1[:],
                    op0=AT.mult, op1=AT.mult,
                )
                c2 = smallp.tile([P, T], f32, tag="c2", name="c2")
                nc.vector.scalar_tensor_tensor(
                    out=c2[:], in0=c1[:], scalar=beta1, in1=s1p[:],
                    op0=AT.add, op1=AT.add,
                )
                s2p = smallp.tile([P, T], f32, tag="s2p", name="s2p")
                nc.vector.scalar_tensor_tensor(
                    out=s2p[:], in0=a2, scalar=1.0, in1=c2[:],
                    op0=AT.mult, op1=AT.mult,
                )
                c3blk = smallp.tile([P, T], f32, tag="c3b", name="c3blk")
                nc.vector.scalar_tensor_tensor(
                    out=c3blk[:], in0=c2[:], scalar=beta2, in1=s2p[:],
                    op0=AT.add, op1=AT.add,
                )
                c3blks[g] = c3blk

            def stage_bcast(g):
                B = BLKS[g]
                T = B // P
                c3blk = c3blks.pop(g)
                psc3T = psCp.tile([1, B], f32, tag="c3t", name="psc3T")
                for t in range(T):
                    nc.tensor.transpose(
                        psc3T[0:1, t * P : (t + 1) * P],
                        c3blk[:, t : t + 1],
                        ident128,
                    )
                c3Ts = smallp.tile([1, B], f32, tag="c3ts", name="c3Ts")
                nc.scalar.copy(c3Ts[:], psc3T[:])
                psR = psRp.tile([P, B], f32, tag="c3rep", name="psR")
                for t in range(T):
                    nc.tensor.matmul(
                        psR[:, t * P : (t + 1) * P],
                        onesf[0:1, :],
                        c3Ts[0:1, t * P : (t + 1) * P],
                        start=True,
                        stop=True,
                    )
                c3rep = c3rp.tile([P, B], bf16, tag="c3repb", name="c3rep")
                nc.scalar.copy(c3rep[:], psR[:])
                c3reps[g] = c3rep

            def stage_fin_mult(g):
                B = BLKS[g]
                xtb = xtbs[g]
                c3rep = c3reps.pop(g)
                tmp = tmpp.tile([P, NCH * B], bf16, tag="t", name="tmp")
                c3bc = (
                    c3rep[:]
                    .rearrange("p (o b) -> p o b", o=1)
                    .broadcast_to([P, NCH, B])
                )
                nc.vector.tensor_tensor(
                    out=tmp[:].rearrange("p (k b) -> p k b", k=NCH),
                    in0=xtb[:].rearrange("p (k b) -> p k b", k=NCH),
                    in1=c3bc,
                    op=AT.mult,
                )
                tmps[g] = tmp

            def stage_fin_add(g):
                B = BLKS[g]
                xtbs.pop(g)
                tmp = tmps.pop(g)
                otb = otp.tile([P, NCH * B], bf16, name="otb")
                ov = otb[:].rearrange("p (h k b) -> h p k b", h=2, b=B)
                dst = io_view(outt_d, g)
                b3v = b3rep[:].rearrange("p (k b) -> p k b", k=NCH)
                nc.vector.tensor_tensor(
                    out=otb[:].rearrange("p (k b) -> p k b", k=NCH)[:, 0:4],
                    in0=tmp[:].rearrange("p (k b) -> p k b", k=NCH)[:, 0:4],
                    in1=b3v[:, 0:4, 0:B],
                    op=AT.add,
                )
                nc.scalar.dma_start(dst[0], ov[0])
                for k in (4, 5, 6):
                    nc.scalar.activation(
                        otb[:, k * B : (k + 1) * B],
                        tmp[:, k * B : (k + 1) * B],
                        AF.Identity,
                        bias=b3cols[:, k : k + 1],
                    )
                nc.vector.tensor_scalar_add(
                    otb[:, 7 * B : 8 * B],
                    tmp[:, 7 * B : 8 * B],
                    b3cols[:, 7:8],
                )
                nc.scalar.dma_start(dst[1], ov[1])

            nb = len(BLKS)
            stage_load(0)
            stage_load(1)
            stage_dots(0)
            stage_rec(0)
            for g in range(nb):
                if g + 2 < nb:
                    stage_load(g + 2)
                stage_bcast(g)
                if g + 1 < nb:
                    stage_dots(g + 1)
                    stage_rec(g + 1)
                stage_fin_mult(g)
                stage_fin_add(g)

    nc.compile()
    return nc


def _prep_tp_inputs(x, kernels, bias):
    import ml_dtypes

    bf = ml_dtypes.bfloat16
    x_bf = np.ascontiguousarray(x, dtype=np.float32).astype(bf)
    W = np.ascontiguousarray(kernels[:, :, 0], dtype=np.float32)
    Bb = np.ascontiguousarray(bias[:, :, 0], dtype=np.float32)
    beta1 = float(Bb[0] @ W[1])
    beta2 = float((Bb[0] + Bb[1]) @ W[2])
    b3 = (Bb[0] + Bb[1] + Bb[2]).astype(np.float32)

    wck = np.zeros((P, 8, 4), dtype=np.float32)
    wck[:, :, 0:3] = W.T.reshape(8, P, 3).transpose(1, 0, 2)
    prmb = wck.reshape(P, 32).astype(bf)

    prmf = np.zeros((P, 140), dtype=np.float32)
    prmf[:, 0:128] = np.eye(P, dtype=np.float32)
    prmf[0:3, 128:131] = np.eye(3, dtype=np.float32)
    prmf[:, 132:140] = b3.reshape(8, P).T
    return x_bf, prmb, prmf, beta1, beta2


def _prep_b16_inputs(x, kernels, bias):
    import ml_dtypes

    bf = ml_dtypes.bfloat16
    x_bf = np.ascontiguousarray(x, dtype=np.float32).astype(bf)
    W = np.ascontiguousarray(kernels[:, :, 0], dtype=np.float32)
    Bb = np.ascontiguousarray(bias[:, :, 0], dtype=np.float32)
    beta1 = float(Bb[0] @ W[1])
    beta2 = float((Bb[0] + Bb[1]) @ W[2])
    b3 = (Bb[0] + Bb[1] + Bb[2]).astype(np.float32)

    auxb = np.zeros((P, 160), dtype=np.float32)
    auxb[:, 0:128] = np.eye(P, dtype=np.float32)
    wsb4 = np.zeros((P, 8, 4), dtype=np.float32)
    wsb4[:, :, 0:3] = W.T.reshape(8, P, 3).transpose(1, 0, 2)
    auxb[:, 128:160] = wsb4.reshape(P, 32)
    auxb = auxb.astype(bf)
    b3row = b3.reshape(1, F).astype(bf)
    w3row = W.astype(bf)
    betas = np.zeros((P, 2), dtype=np.float32)
    betas[:, 0] = beta1
    betas[:, 1] = beta2
    return x_bf, auxb, b3row, w3row, betas, beta1, beta2


import os

VERSION = os.environ.get("KERNEL_V", "tp")
N_PE = int(os.environ.get("KERNEL_NPE", "6"))
N_BLOCKS = int(os.environ.get("KERNEL_NBLK", "4"))


def _get_nc_and_inmaps(x, kernels, bias):
    if VERSION == "tp":
        x_bf, prmb, prmf, beta1, beta2 = _prep_tp_inputs(x, kernels, bias)
        key = (VERSION, N_BLOCKS, beta1, beta2)
        if key not in _compiled:
            _compiled[key] = _build_tp(beta1, beta2, n_blocks=N_BLOCKS)
        nc = _compiled[key]
        in_maps = [
            {
                "xt": np.ascontiguousarray(
                    x_bf[c * B_CORE : (c + 1) * B_CORE, :].T
                ),
                "prmb": prmb,
                "prmf": prmf,
            }
            for c in range(N_CORES)
        ]
        return nc, in_maps
    if VERSION == "b16":
        x_bf, auxb, b3row, w3row, betas, beta1, beta2 = _prep_b16_inputs(
            x, kernels, bias
        )
        key = (VERSION, N_PE, beta1, beta2)
        if key not in _compiled:
            _compiled[key] = _build_b16(beta1, beta2, n_pe=N_PE)
        nc = _compiled[key]
        in_maps = [
            {
                "x": x_bf[c * B_CORE : (c + 1) * B_CORE],
                "auxb": auxb,
                "b3row": b3row,
                "w3row": w3row,
                "betas": betas,
            }
            for c in range(N_CORES)
        ]
        return nc, in_maps
    x, aux, b3row, beta1, beta2 = _prep_pe_inputs(x, kernels, bias)
    key = (VERSION, N_PE, beta1, beta2)
    if key not in _compiled:
        if VERSION == "h2":
            _compiled[key] = _build_h2(beta1, beta2, n_pe=N_PE)
        elif VERSION == "hybrid":
            _compiled[key] = _build_hybrid(beta1, beta2, n_pe=N_PE)
        elif VERSION == "pe":
            _compiled[key] = _build_pe(beta1, beta2)
        else:
            _compiled[key] = _build(beta1, beta2)
    nc = _compiled[key]
    if VERSION == "h2":
        W = np.ascontiguousarray(kernels[:, :, 0], dtype=np.float32)
        in_maps = [
            {
                "x": x[c * B_CORE : (c + 1) * B_CORE],
                "aux": aux,
                "b3row": b3row,
                "w3row": W,
            }
            for c in range(N_CORES)
        ]
    elif VERSION == "hybrid":
        W = np.ascontiguousarray(kernels[:, :, 0], dtype=np.float32)
        wrep = np.broadcast_to(
            np.concatenate([W[0], W[1], W[2]]), (P, 3 * F)
        ).copy()
        in_maps = [
            {
                "x": x[c * B_CORE : (c + 1) * B_CORE],
                "aux": aux,
                "b3row": b3row,
                "wrep": wrep,
            }
            for c in range(N_CORES)
        ]
    elif VERSION == "pe":
        in_maps = [
            {"x": x[c * B_CORE : (c + 1) * B_CORE], "aux": aux, "b3row": b3row}
            for c in range(N_CORES)
        ]
    else:
        W = np.ascontiguousarray(kernels[:, :, 0], dtype=np.float32)
        Bb = np.ascontiguousarray(bias[:, :, 0], dtype=np.float32)
        b3 = Bb[0] + Bb[1] + Bb[2]
        wb = np.concatenate([W[0], W[1], W[2], b3]).astype(np.float32)
        wb = np.broadcast_to(wb, (P, 4 * F)).copy()
        in_maps = [
            {"x": x[c * B_CORE : (c + 1) * B_CORE], "wb": wb}
            for c in range(N_CORES)
        ]
    return nc, in_maps


def kernel(x: np.ndarray, kernels: np.ndarray, bias: np.ndarray) -> np.ndarray:
    nc, in_maps = _get_nc_and_inmaps(x, kernels, bias)
    last_err = None
    for _attempt in range(3):
        try:
            res = run_bass_kernel_spmd(nc, in_maps, core_ids=list(range(N_CORES)))
            break
        except Exception as e:
            last_err = e
    else:
        raise last_err
    if VERSION == "tp":
        outt = np.concatenate(
            [np.asarray(res.results[c]["outt"]) for c in range(N_CORES)], axis=1
        )
        return outt.T.astype(np.float32)
    out = np.concatenate(
        [np.asarray(res.results[c]["out"]) for c in range(N_CORES)], axis=0
    )
    return out.astype(np.float32)


def timed_run(x, kernels, bias):
    nc, in_maps = _get_nc_and_inmaps(x, kernels, bias)
    res = run_bass_kernel_spmd(
        nc, in_maps, core_ids=list(range(N_CORES)), trace=True
    )
    print(
        "exec_time_ns:", res.exec_time_ns,
        "mean:", res.mean_exec_time_ns,
        "max core:", res.max_exec_time_core_id,
    )
    if res.instructions_and_trace:
        print("trace:", res.instructions_and_trace[1])
    return res.exec_time_ns


if __name__ == "__main__":
    rng = np.random.default_rng(0)
    x = rng.standard_normal((B_FULL, F), dtype=np.float32)
    k = rng.standard_normal((3, F, 1), dtype=np.float32) * 0.07
    b = rng.standard_normal((3, F, 1), dtype=np.float32) * 0.07
    out = kernel(x=x, kernels=k, bias=b)
    print("out", out.shape, out.dtype)

